# revision 40
# baseline (speedup 1.0000x reference)
"""Trainium2 Bass kernel for packed-segment causal GQA attention.

Shapes (hardcoded): x [4096, 2048], 16 q heads / 4 kv heads, head dim 128,
4 packed segments of 1024 tokens.

Sharding over 8 cores: core c -> segment c//2 (tokens), head-half c%2
(8 q heads + their 2 kv heads; wq/wk/wv column-sharded, wo row-sharded).
Each core computes a partial out^T [2048, 1024] for its segment; host sums
the two partials per segment (wo row-shard) and transposes back.

Optimizations vs the 244us bf16 baseline (now ~205us, PE ~90% busy):
- p^T = exp(s) stored fp8e4 (the only tensor whose quantization noise fits
  the error budget - weight-side fp8 noise does NOT average out through
  attention; measured ablation puts any weight-side fp8 at 1.5-3.9% relmax
  vs the 2% gate).  The softmax-denominator matmul then runs as fp8
  DoubleRow over PAIRED key blocks (half the PE streaming) while attn@V
  consumes the same fp8 p^T against a bf16 stationary v (mixed-dtype
  matmul).  Projections/scores/attn@V stay bf16.
- K and the first 4 V token blocks are projected chunk-major, riding the
  incoming xT DMA stream (per-chunk loads interleaved wk/wv/xT).  One
  PSUM bank per accumulation group: matmul start=True clears the whole
  bank's has_written bits, so groups must never share a bank.
- RoPE pair-swap runs as a DVE stream_shuffle (mask p^1 within 32-lane
  groups) instead of a PE permutation matmul; rope muls/adds are bf16.
- Head loop split into an rg0 sweep (rows 0-511, q-proj of head h+1
  woven in) and an rg1 sweep woven with the tb=0 out-projection chunks,
  so the PE never waits on ACT exps; warm-up matmuls on the memset ones
  tile release the HAM clock throttle during the first input DMAs.
- Output DMA'd bf16 (halved), summed across the wo row-shard pair in
  fp32 on host.  1/denom via exp(-ln(d)) on ACT (DVE reciprocal too slow,
  custom fast-approx DVE op doesn't compile on this walrus).

On-device dataflow (all in transposed token space, fp32 PSUM):
  q^T = wq_sh^T @ x_seg^T            (lhsT = wq_sh, rhs = x^T, bf16)
  RoPE via pair-swap matmul + cos/sin elementwise combine (bf16)
  s^T[key, row] = k^T_tile^T.T @ q^T  per 128-key x 512-row block
  p^T = exp(s^T/sqrt(d)) -> fp8, additive triangular mask on diag chunks
  denom = ones^T @ p^T  (fp8 DoubleRow over paired key blocks)
  o^T += v_tile.T @ p^T (v bf16 stationary, p^T fp8 moving)
  a^T = o^T * exp(-ln(denom));  out^T = wo_sh^T @ a^T (bf16)
"""

import os
import re

import numpy as np
import ml_dtypes

import bass_rust
import concourse.bass as bass
import concourse.mybir as mybir
import concourse.tile as tile
from concourse.bass_utils import run_bass_kernel_spmd
from concourse.vector_clock import ScopedClock

BF16 = ml_dtypes.bfloat16
F32 = mybir.dt.float32
BF = mybir.dt.bfloat16
FP8 = mybir.dt.float8e4
DR = mybir.MatmulPerfMode.DoubleRow

DIM, H, HKV, D, B, S = 2048, 16, 4, 128, 4, 1024
REP = H // HKV
SCALE = float(D) ** -0.5
NCORE = 8
HLOC = 8          # q heads per core
GLOC = 2          # kv heads per core
KC = DIM // 128   # 16 contraction chunks
NTB = S // 512    # 2 row blocks of 512
NKB = S // 128    # 8 key blocks of 128


_MAXW = 1


def _patch_wait_split(tilemod):
    """walrus in this env caps sem waits per instruction: rewrite any
    instruction carrying more than _MAXW waits so the excess waits land on
    same-engine NoOps inserted just before it."""

    orig_lower = tilemod.TileContext._lower_ordered_insts

    def _split_block(nc, insts):
        out = []
        for inst in insts:
            si = getattr(inst, "sync_info", None)
            waits = list(si.on_wait) if si is not None and si.on_wait else []
            if len(waits) > _MAXW:
                head, rest = waits[:-_MAXW], waits[-_MAXW:]
                for w in head:  # NoOp is CTRL-class: keep it to 1 wait each
                    out.append(
                        mybir.InstNoOp(
                            name=nc.get_next_instruction_name(),
                            engine=inst.engine,
                            bass_nofuse=True,
                            debug=inst.debug,
                            sync_info=mybir.SyncInfo(on_wait=[w], on_update=[]),
                        )
                    )
                inst.sync_info = mybir.SyncInfo(
                    on_wait=rest, on_update=list(si.on_update)
                )
            out.append(inst)
        insts[:] = out

    def patched(self, ordered):
        for insts in ordered.values():
            _split_block(self.nc, insts)
        return orig_lower(self, ordered)

    tilemod.TileContext._lower_ordered_insts = patched


def _patch_drain(tilemod):
    """walrus in this env rejects >1 sem wait on CTRL instructions: split the
    TileContext-exit drain's waits across single-wait SP NoOPs."""

    def _drain_and_barrier_split(self, tick_clock, wait_clock):
        nc = self.nc
        gc = tick_clock.global_clock
        ticks = [int(t) for t in re.findall(r"\d+", str(gc))]
        for idx, tick in enumerate(ticks):
            if tick <= 0:
                continue
            part = bass_rust.VectorClock()
            part.require_at_least(idx, tick)
            n = nc.sync.nop(hint="drain_split", nofuse=True)
            wait_clock.add_sem_waits(n.ins, ScopedClock({None: part}))
        d = nc.sync.drain()
        wait_clock.add_sem_waits(
            d.ins, ScopedClock({None: gc}), cur_clock=ScopedClock({None: gc})
        )
        nc.all_engine_barrier()
        assert self.sems is not None
        popped = nc._tile_sem_poison_stack.pop()
        assert popped is self._sem_poison
        nc.clear_and_free_semaphores(list(self.sems.allocated().values()))
        nc.all_engine_barrier()

    tilemod.TileContext._drain_and_barrier = _drain_and_barrier_split


_patch_wait_split(tile)
_patch_drain(tile)

_PROGRAM = None


def _build_program():
    nc = bass.Bass()

    xT = nc.declare_dram_parameter("xT", [DIM, S], BF, isOutput=False)
    wq = nc.declare_dram_parameter("wq", [DIM, HLOC * D], BF, isOutput=False)
    wk = nc.declare_dram_parameter("wk", [DIM, GLOC * D], BF, isOutput=False)
    wv = nc.declare_dram_parameter("wv", [DIM, GLOC * D], BF, isOutput=False)
    wo = nc.declare_dram_parameter("wo", [HLOC * D, DIM], BF, isOutput=False)
    cosT = nc.declare_dram_parameter("cosT", [D, S], BF, isOutput=False)
    sinT = nc.declare_dram_parameter("sinT", [D, S], BF, isOutput=False)
    tri = nc.declare_dram_parameter("tri", [D, D], F32, isOutput=False)
    outT = nc.declare_dram_parameter("outT", [DIM, S], BF, isOutput=True)

    # rope pair-swap (partition p <-> p^1) as a 32-lane DVE stream shuffle
    swap_mask = [i ^ 1 for i in range(32)]

    xT_r = xT.rearrange("(o p) t -> p o t", p=128)      # [128, 16, 1024]
    wq_r = wq.rearrange("(o p) f -> p o f", p=128)      # [128, 16, 1024]
    wk_r = wk.rearrange("(o p) f -> p o f", p=128)      # [128, 16, 256]
    wv_r = wv.rearrange("(o p) f -> p o f", p=128)      # [128, 16, 256]
    wo_r = wo.rearrange("(c p) e -> p c e", p=128)      # [128, 8, 2048]
    outT_r = outT.rearrange("(o p) t -> p o t", p=128)  # [128, 16, 1024]

    with tile.TileContext(nc) as tc:
        with (
            tc.tile_pool(name="consts", bufs=1) as consts,
            tc.tile_pool(name="rope", bufs=2) as rope_pool,
            tc.tile_pool(name="dinv", bufs=2) as dinv_pool,
            tc.tile_pool(name="ostage", bufs=3) as ostage,
            tc.tile_pool(name="psA", bufs=2, space="PSUM") as psA,
            tc.tile_pool(name="psS", bufs=3, space="PSUM") as psS,
            tc.tile_pool(name="psO", bufs=2, space="PSUM") as psO,
            tc.tile_pool(name="psD", bufs=1, space="PSUM") as psD,
        ):
            # ---- constant loads (ordered so K proj starts ~7us in and the
            # chunk-major K matmuls ride the incoming xT stream) ----
            xT_sb = consts.tile([128, KC, S], BF)
            wk_sb = consts.tile([128, KC, GLOC * D], BF)
            wq_sb = consts.tile([128, KC, HLOC * D], BF)
            wv_sb = consts.tile([128, KC, GLOC * D], BF)
            nc.sync.dma_start(out=wk_sb[:, 0, :], in_=wk_r[:, 0, :])
            nc.sync.dma_start(out=xT_sb[:, 0, 0:512], in_=xT_r[:, 0, 0:512])
            nc.sync.dma_start(out=wv_sb[:, 0, :], in_=wv_r[:, 0, :])
            nc.sync.dma_start(out=xT_sb[:, 0, 512:1024],
                              in_=xT_r[:, 0, 512:1024])
            nc.sync.dma_start(out=wk_sb[:, 1:4, :], in_=wk_r[:, 1:4, :])
            nc.sync.dma_start(out=wv_sb[:, 1:4, :], in_=wv_r[:, 1:4, :])
            nc.sync.dma_start(out=xT_sb[:, 1, :], in_=xT_r[:, 1, :])
            nc.sync.dma_start(out=xT_sb[:, 2, :], in_=xT_r[:, 2, :])
            nc.sync.dma_start(out=wk_sb[:, 4:8, :], in_=wk_r[:, 4:8, :])
            nc.sync.dma_start(out=wv_sb[:, 4:8, :], in_=wv_r[:, 4:8, :])
            nc.sync.dma_start(out=xT_sb[:, 3, :], in_=xT_r[:, 3, :])
            nc.sync.dma_start(out=xT_sb[:, 4, :], in_=xT_r[:, 4, :])
            nc.sync.dma_start(out=wk_sb[:, 8:12, :], in_=wk_r[:, 8:12, :])
            nc.sync.dma_start(out=wv_sb[:, 8:12, :], in_=wv_r[:, 8:12, :])
            nc.sync.dma_start(out=xT_sb[:, 5, :], in_=xT_r[:, 5, :])
            nc.sync.dma_start(out=xT_sb[:, 6:8, :], in_=xT_r[:, 6:8, :])
            nc.sync.dma_start(out=wk_sb[:, 12:16, :], in_=wk_r[:, 12:16, :])
            nc.sync.dma_start(out=wv_sb[:, 12:16, :], in_=wv_r[:, 12:16, :])
            nc.sync.dma_start(out=xT_sb[:, 8:10, :], in_=xT_r[:, 8:10, :])
            nc.sync.dma_start(out=xT_sb[:, 10:12, :], in_=xT_r[:, 10:12, :])
            nc.sync.dma_start(out=xT_sb[:, 12:14, :], in_=xT_r[:, 12:14, :])
            nc.sync.dma_start(out=xT_sb[:, 14:16, :], in_=xT_r[:, 14:16, :])
            cos_sb = consts.tile([128, S], BF)
            nc.sync.dma_start(out=cos_sb, in_=cosT[:, :])
            sin_sb = consts.tile([128, S], BF)
            nc.sync.dma_start(out=sin_sb, in_=sinT[:, :])
            tri_sb = consts.tile([128, D], F32)
            nc.sync.dma_start(out=tri_sb, in_=tri[:, :])
            nc.sync.dma_start(out=wq_sb[:, 0:8, :], in_=wq_r[:, 0:8, :])
            nc.sync.dma_start(out=wq_sb[:, 8:16, :], in_=wq_r[:, 8:16, :])
            wo_sb = consts.tile([128, HLOC, DIM], BF)
            nc.sync.dma_start(out=wo_sb[:, :, :], in_=wo_r[:, :, :])

            # all-ones stationary operand (fp8 pair layout): the DoubleRow
            # denom matmul writes the key-sum to EVERY output partition
            ones2 = consts.tile([128, 2, 128], FP8)
            nc.vector.memset(ones2, 1.0)

            # warm-up matmuls on the memset tile while the first input DMAs
            # are in flight: releases the HAM clock throttle (K=4/8 ->
            # 8/8) before the real matmuls arrive, at zero DMA cost
            for w in range(10):
                wps = psO.tile([128, 512], F32, tag="psO")
                nc.tensor.matmul(
                    wps[:, 0:256],
                    ones2[:, 0, :],
                    ones2[:, :, :],
                    start=True,
                    stop=True,
                )

            # persistent activations
            qT_sb = consts.tile([128, HLOC, S], BF)   # q^T, rotated
            kT_sb = consts.tile([128, GLOC, S], BF)   # k^T, rotated
            v_sb = consts.tile([128, NKB, GLOC * D], BF)  # v in [tok, d]
            aT_sb = consts.tile([128, HLOC, S], BF)   # attention out^T

            # persistent p^T tiles (fp8), two-deep per row-group so head h+1's
            # exp can overlap head h's attnV.  The never-exp-written "gap"
            # columns of odd diagonal blocks are zeroed once and stay 0, so
            # the paired DoubleRow denom matmuls read 0 there.
            pt0 = [
                consts.tile([128, 4, 512], FP8, name=f"pt0_{i}")
                for i in range(2)
            ]
            pt1 = [
                consts.tile([128, NKB, 512], FP8, name=f"pt1_{i}")
                for i in range(2)
            ]
            for t in pt0:
                nc.vector.memset(t[:, 1, 0:128], 0.0)
                nc.vector.memset(t[:, 3, 256:384], 0.0)
            for t in pt1:
                nc.vector.memset(t[:, 5, 0:128], 0.0)
                nc.vector.memset(t[:, 7, 256:384], 0.0)

            def rope_combine(ps, tb, dst_sb, dst_idx):
                """dst[:, dst_idx, tb*512:+512] = rope(ps) via DVE shuffle."""
                qsb = rope_pool.tile([128, 512], BF, tag="qsb")
                nc.scalar.copy(out=qsb, in_=ps)
                qsw = rope_pool.tile([128, 512], BF, tag="qsw")
                nc.vector.stream_shuffle(out=qsw, in_=qsb, mask=swap_mask)
                tspan = slice(tb * 512, (tb + 1) * 512)
                t1 = rope_pool.tile([128, 512], BF, tag="t1")
                nc.vector.tensor_mul(out=t1, in0=qsb, in1=cos_sb[:, tspan])
                t2 = rope_pool.tile([128, 512], BF, tag="t2")
                nc.vector.tensor_mul(out=t2, in0=qsw, in1=sin_sb[:, tspan])
                nc.vector.tensor_add(
                    out=dst_sb[:, dst_idx, tspan], in0=t1, in1=t2
                )

            def proj_rope(w_sb, hd_idx, tb, dst_sb, dst_idx):
                """dst[:, dst_idx, tb*512:+512] = rope(w^T @ x^T) tile."""
                ps = psA.tile([128, 512], F32, tag="ps")
                for kc in range(KC):
                    nc.tensor.matmul(
                        ps,
                        w_sb[:, kc, hd_idx * 128:(hd_idx + 1) * 128],
                        xT_sb[:, kc, tb * 512:(tb + 1) * 512],
                        start=(kc == 0),
                        stop=(kc == KC - 1),
                    )
                rope_combine(ps, tb, dst_sb, dst_idx)

            # ---- K + V projections, chunk-major so the accumulations ride
            # the incoming xT chunk stream instead of head-of-line blocking.
            # V packs two token blocks per [128,512] PSUM tile. ----
            kacc = []
            for g in range(GLOC):
                for tb in range(NTB):
                    pool = psA if g == 0 else psO
                    kacc.append(pool.tile([128, 512], F32,
                                          tag="ps" if g == 0 else "psO",
                                          name=f"kacc{g}{tb}"))
            # v accumulators for token blocks 0-3 ride the xT stream too.
            # One PSUM tile per block: a matmul's start=True clears the WHOLE
            # bank's has_written bits, so two interleaved accumulation groups
            # must never share a bank.
            vacc = [
                psS.tile([128, 512], F32, tag="psS", name=f"vacc{i}")
                for i in range(3)
            ] + [psD.tile([128, 512], F32, tag="psD", name="vacc3")]
            # per chunk: tb=0 K matmuls and v-blocks 0-3 only touch the first
            # 512 columns, so they run while the chunk's second half streams
            kidx = {(g, tb): i for i, (g, tb) in enumerate(
                (g, tb) for g in range(GLOC) for tb in range(NTB)
            )}
            for kc in range(KC):
                for g, tb in ((0, 0), (1, 0)):
                    nc.tensor.matmul(
                        kacc[kidx[(g, tb)]],
                        wk_sb[:, kc, g * 128:(g + 1) * 128],
                        xT_sb[:, kc, tb * 512:(tb + 1) * 512],
                        start=(kc == 0),
                        stop=(kc == KC - 1),
                    )
                for vtb in range(4):
                    nc.tensor.matmul(
                        vacc[vtb][:, : GLOC * D],
                        xT_sb[:, kc, vtb * 128:(vtb + 1) * 128],
                        wv_sb[:, kc, :],
                        start=(kc == 0),
                        stop=(kc == KC - 1),
                    )
                for g, tb in ((0, 1), (1, 1)):
                    nc.tensor.matmul(
                        kacc[kidx[(g, tb)]],
                        wk_sb[:, kc, g * 128:(g + 1) * 128],
                        xT_sb[:, kc, tb * 512:(tb + 1) * 512],
                        start=(kc == 0),
                        stop=(kc == KC - 1),
                    )
            for i, (g, tb) in enumerate(
                (g, tb) for g in range(GLOC) for tb in range(NTB)
            ):
                rope_combine(kacc[i], tb, kT_sb, g)
            for vtb in range(4):
                nc.scalar.copy(out=v_sb[:, vtb, :], in_=vacc[vtb][:, : GLOC * D])
            for vtb in range(4, NKB):
                ps = psA.tile([128, 512], F32, tag="ps")
                for kc in range(KC):
                    nc.tensor.matmul(
                        ps[:, : GLOC * D],
                        xT_sb[:, kc, vtb * 128:(vtb + 1) * 128],
                        wv_sb[:, kc, :],
                        start=(kc == 0),
                        stop=(kc == KC - 1),
                    )
                nc.scalar.copy(out=v_sb[:, vtb, :], in_=ps[:, : GLOC * D])

            def score_block(h, rg, kb, pt):
                """scores matmul + mask + exp -> pt for one 128-key block."""
                g = h // REP
                c0 = max(0, kb - 4 * rg)
                span = slice(c0 * 128, 512)
                ps = psS.tile([128, 512], F32, tag="psS")
                nc.tensor.matmul(
                    ps[:, span],
                    kT_sb[:, g, kb * 128:(kb + 1) * 128],
                    qT_sb[:, h, rg * 512 + c0 * 128:(rg + 1) * 512],
                    start=True,
                    stop=True,
                )
                if kb - 4 * rg >= 0:
                    cc = kb - 4 * rg
                    nc.vector.tensor_add(
                        out=ps[:, cc * 128:(cc + 1) * 128],
                        in0=ps[:, cc * 128:(cc + 1) * 128],
                        in1=tri_sb,
                    )
                nc.scalar.activation(
                    out=pt[:, kb, span],
                    in_=ps[:, span],
                    func=mybir.ActivationFunctionType.Exp,
                    scale=SCALE,
                )

            def consume_block(h, rg, kb, pt, po, pd):
                """attn@V for block kb (+ paired DoubleRow denom on odd kb)."""
                g = h // REP
                nkb = 4 * rg + 4
                c0 = max(0, kb - 4 * rg)
                span = slice(c0 * 128, 512)
                # attn@V per block: bf16 v stationary, fp8 p^T moving
                nc.tensor.matmul(
                    po[:, span],
                    v_sb[:, kb, g * D:(g + 1) * D],
                    pt[:, kb, span],
                    start=(kb == 0),
                    stop=(kb == nkb - 1),
                )
                if kb % 2 == 1:
                    p0 = kb - 1
                    c00 = max(0, p0 - 4 * rg)
                    spanp = slice(c00 * 128, 512)
                    nc.tensor.matmul(
                        pd[:, spanp],
                        ones2,
                        pt[:, p0:p0 + 2, spanp],
                        start=(p0 == 0),
                        stop=(p0 == nkb - 2),
                        perf_mode=DR,
                    )

            def normalize(h, rg, po, pd):
                # 1/denom as exp(-ln(denom)) on ACT: DVE reciprocal costs
                # ~6.5ns/elem and the fast-approx custom op doesn't
                # compile on this walrus ("ISA wrong length")
                rows = slice(rg * 512, (rg + 1) * 512)
                lnd = dinv_pool.tile([128, 512], F32, tag="lnd")
                nc.scalar.activation(
                    out=lnd, in_=pd, func=mybir.ActivationFunctionType.Ln
                )
                dinv_b = dinv_pool.tile([128, 512], F32, tag="dinvb")
                nc.scalar.activation(
                    out=dinv_b, in_=lnd,
                    func=mybir.ActivationFunctionType.Exp, scale=-1.0,
                )
                nc.vector.tensor_mul(
                    out=aT_sb[:, h, rows],
                    in0=po,
                    in1=dinv_b,
                )

            def attention_rg(h, rg):
                pt = (pt0 if rg == 0 else pt1)[h % 2]
                po = psO.tile([128, 512], F32, tag="psO")
                pd = psD.tile([128, 512], F32, tag="psD")
                for kb in range(4 * rg + 4):
                    score_block(h, rg, kb, pt)
                    consume_block(h, rg, kb, pt, po, pd)
                normalize(h, rg, po, pd)

            def op_group(et, tb):
                """one [128, 512] chunk of out^T = wo_sh^T @ a^T."""
                ps = psA.tile([128, 512], F32, tag="ps")
                for c in range(HLOC):
                    nc.tensor.matmul(
                        ps,
                        wo_sb[:, c, et * 128:(et + 1) * 128],
                        aT_sb[:, c, tb * 512:(tb + 1) * 512],
                        start=(c == 0),
                        stop=(c == HLOC - 1),
                    )
                st = ostage.tile([128, 512], BF, tag="st")
                nc.vector.tensor_copy(out=st, in_=ps)
                nc.sync.dma_start(
                    out=outT_r[:, et, tb * 512:(tb + 1) * 512], in_=st
                )

            # ---- rg0 sweep: project q for head h+1, attend head h rows
            # 0-511 (q-proj matmuls absorb the ACT exp latency) ----
            for tb in range(NTB):
                proj_rope(wq_sb, 0, tb, qT_sb, 0)
            for h in range(HLOC):
                if h + 1 < HLOC:
                    for tb in range(NTB):
                        proj_rope(wq_sb, h + 1, tb, qT_sb, h + 1)
                attention_rg(h, 0)

            # ---- rg1 sweep woven with tb=0 out-proj chunks (all heads'
            # rows 0-511 of a^T are done, so those 16 chunks keep the PE
            # busy while ACT works through the rg1 exps) ----
            for h in range(HLOC):
                if h > 0:
                    op_group(2 * h - 2, 0)
                    op_group(2 * h - 1, 0)
                attention_rg(h, 1)
            # the trailing tb=0 chunks absorb head 7's ACT/DVE drain before
            # the tb=1 out-proj sweep begins
            op_group(KC - 2, 0)
            op_group(KC - 1, 0)
            for et in range(KC - 1):
                op_group(et, 1)
            # final chunk as two half-width accumulation groups (separate
            # PSUM buffers!) so the first half's staging copy + DMA overlap
            # the second half's matmuls instead of trailing them
            for half in range(2):
                ph = psA.tile([128, 512], F32, tag="ps", name=f"ps_l{half}")
                cols = slice(512 + half * 256, 512 + (half + 1) * 256)
                for c in range(HLOC):
                    nc.tensor.matmul(
                        ph[:, 0:256],
                        wo_sb[:, c, (KC - 1) * 128:KC * 128],
                        aT_sb[:, c, cols],
                        start=(c == 0),
                        stop=(c == HLOC - 1),
                    )
                sth = ostage.tile([128, 256], BF, tag="sth", name=f"sth{half}")
                nc.vector.tensor_copy(out=sth, in_=ph[:, 0:256])
                nc.sync.dma_start(out=outT_r[:, KC - 1, cols], in_=sth)

    return nc


LAST_RESULT = None
_TRACE = os.environ.get("BASS_ATTN_TRACE", "") == "1"

if _TRACE:
    # Register the NTFF profile hook that the agent image's antenv lacks
    # (test/profiling only; the graded path never enters this branch).
    try:
        import sys
        import types

        import antenv  # noqa: F401

        if "antenv.axon_hooks" not in sys.modules:
            _mod = types.ModuleType("antenv.axon_hooks")
            _hook_box = [None]
            _mod.set_axon_ntff_profile_hook = lambda h: _hook_box.__setitem__(0, h)
            _mod.get_axon_ntff_profile_hook = lambda: _hook_box[0]
            sys.modules["antenv.axon_hooks"] = _mod
            import antenv as _antenv

            _antenv.axon_hooks = _mod
            from trn_agent_boot.trn_boot import _ntff_profile_via_ctypes

            _mod.set_axon_ntff_profile_hook(
                _ntff_profile_via_ctypes("/opt/axon/libaxon_pjrt.so")
            )
    except Exception as e:  # pragma: no cover
        print(f"NTFF hook setup failed ({e}); tracing will be skipped")


def kernel(x, freqs_cis, wq, wk, wv, wo, seq_len=None, **_ignored):
    global _PROGRAM, LAST_RESULT
    x = np.ascontiguousarray(np.asarray(x, dtype=np.float32))
    fc = np.asarray(freqs_cis, dtype=np.float32)
    wq = np.asarray(wq, dtype=np.float32)
    wk = np.asarray(wk, dtype=np.float32)
    wv = np.asarray(wv, dtype=np.float32)
    wo = np.asarray(wo, dtype=np.float32)

    # host-side prep (sharding + transposed/bf16 views + rope/mask constants)
    xT = np.ascontiguousarray(x.T).astype(BF16)                    # [2048, 4096]
    cos = np.ascontiguousarray(np.repeat(fc[:S, :, 0], 2, axis=1).T).astype(BF16)
    sgn = np.where(np.arange(D) % 2 == 0, -1.0, 1.0).astype(np.float32)
    sin = np.ascontiguousarray(
        (np.repeat(fc[:S, :, 1], 2, axis=1) * sgn[None, :]).T
    ).astype(BF16)
    k_idx = np.arange(128)[:, None]
    r_idx = np.arange(128)[None, :]
    tri = np.where(r_idx >= k_idx, 0.0, -1e9).astype(np.float32)

    in_maps = []
    for c in range(NCORE):
        s, h2 = c // 2, c % 2
        in_maps.append(
            {
                "xT": np.ascontiguousarray(xT[:, s * S:(s + 1) * S]),
                "wq": wq[:, h2 * HLOC * D:(h2 + 1) * HLOC * D].astype(BF16),
                "wk": wk[:, h2 * GLOC * D:(h2 + 1) * GLOC * D].astype(BF16),
                "wv": wv[:, h2 * GLOC * D:(h2 + 1) * GLOC * D].astype(BF16),
                "wo": wo[h2 * HLOC * D:(h2 + 1) * HLOC * D, :].astype(BF16),
                "cosT": cos,
                "sinT": sin,
                "tri": tri,
            }
        )

    if _PROGRAM is None:
        _PROGRAM = _build_program()

    res = run_bass_kernel_spmd(
        _PROGRAM, in_maps, core_ids=list(range(NCORE)), trace=_TRACE
    )
    LAST_RESULT = res

    out = np.empty((B * S, DIM), np.float32)
    for s in range(B):
        outT = (
            res.results[2 * s]["outT"].astype(np.float32)
            + res.results[2 * s + 1]["outT"].astype(np.float32)
        )
        out[s * S:(s + 1) * S, :] = outT.T
    return out


# revision 42
# speedup vs baseline: 1.0604x; 1.0604x over previous
"""Trainium2 Bass kernel for packed-segment causal GQA attention.

Shapes (hardcoded): x [4096, 2048], 16 q heads / 4 kv heads, head dim 128,
4 packed segments of 1024 tokens.

Sharding over 8 cores: core c -> segment c//2 (tokens), head-half c%2
(8 q heads + their 2 kv heads; wq/wk/wv column-sharded, wo row-sharded).
Each core computes a partial out^T [2048, 1024] for its segment; host sums
the two partials per segment (wo row-shard) and transposes back.

Optimizations vs the 244us bf16 baseline (now ~205us, PE ~90% busy):
- p^T = exp(s) stored fp8e4 (the only tensor whose quantization noise fits
  the error budget - weight-side fp8 noise does NOT average out through
  attention; measured ablation puts any weight-side fp8 at 1.5-3.9% relmax
  vs the 2% gate).  The softmax-denominator matmul then runs as fp8
  DoubleRow over PAIRED key blocks (half the PE streaming) while attn@V
  consumes the same fp8 p^T against a bf16 stationary v (mixed-dtype
  matmul).  Projections/scores/attn@V stay bf16.
- K and the first 4 V token blocks are projected chunk-major, riding the
  incoming xT DMA stream (per-chunk loads interleaved wk/wv/xT).  One
  PSUM bank per accumulation group: matmul start=True clears the whole
  bank's has_written bits, so groups must never share a bank.
- RoPE pair-swap runs as a DVE stream_shuffle (mask p^1 within 32-lane
  groups) instead of a PE permutation matmul; rope muls/adds are bf16.
- Head loop split into an rg0 sweep (rows 0-511, q-proj of head h+1
  woven in) and an rg1 sweep woven with the tb=0 out-projection chunks,
  so the PE never waits on ACT exps; warm-up matmuls on the memset ones
  tile release the HAM clock throttle during the first input DMAs.
- Output DMA'd bf16 (halved), summed across the wo row-shard pair in
  fp32 on host.  1/denom via exp(-ln(d)) on ACT (DVE reciprocal too slow,
  custom fast-approx DVE op doesn't compile on this walrus).

On-device dataflow (all in transposed token space, fp32 PSUM):
  q^T = wq_sh^T @ x_seg^T            (lhsT = wq_sh, rhs = x^T, bf16)
  RoPE via pair-swap matmul + cos/sin elementwise combine (bf16)
  s^T[key, row] = k^T_tile^T.T @ q^T  per 128-key x 512-row block
  p^T = exp(s^T/sqrt(d)) -> fp8, additive triangular mask on diag chunks
  denom = ones^T @ p^T  (fp8 DoubleRow over paired key blocks)
  o^T += v_tile.T @ p^T (v bf16 stationary, p^T fp8 moving)
  a^T = o^T * exp(-ln(denom));  out^T = wo_sh^T @ a^T (bf16)
"""

import os
import re

import numpy as np
import ml_dtypes

import bass_rust
import concourse.bass as bass
import concourse.mybir as mybir
import concourse.tile as tile
from concourse.bass_utils import run_bass_kernel_spmd
from concourse.vector_clock import ScopedClock

BF16 = ml_dtypes.bfloat16
F32 = mybir.dt.float32
BF = mybir.dt.bfloat16
FP8 = mybir.dt.float8e4
DR = mybir.MatmulPerfMode.DoubleRow

DIM, H, HKV, D, B, S = 2048, 16, 4, 128, 4, 1024
REP = H // HKV
SCALE = float(D) ** -0.5
NCORE = 8
HLOC = 8          # q heads per core
GLOC = 2          # kv heads per core
KC = DIM // 128   # 16 contraction chunks
NTB = S // 512    # 2 row blocks of 512
NKB = S // 128    # 8 key blocks of 128


_MAXW = 1


def _patch_wait_split(tilemod):
    """walrus in this env caps sem waits per instruction: rewrite any
    instruction carrying more than _MAXW waits so the excess waits land on
    same-engine NoOps inserted just before it."""

    orig_lower = tilemod.TileContext._lower_ordered_insts

    def _split_block(nc, insts):
        out = []
        for inst in insts:
            si = getattr(inst, "sync_info", None)
            waits = list(si.on_wait) if si is not None and si.on_wait else []
            if len(waits) > _MAXW:
                head, rest = waits[:-_MAXW], waits[-_MAXW:]
                for w in head:  # NoOp is CTRL-class: keep it to 1 wait each
                    out.append(
                        mybir.InstNoOp(
                            name=nc.get_next_instruction_name(),
                            engine=inst.engine,
                            bass_nofuse=True,
                            debug=inst.debug,
                            sync_info=mybir.SyncInfo(on_wait=[w], on_update=[]),
                        )
                    )
                inst.sync_info = mybir.SyncInfo(
                    on_wait=rest, on_update=list(si.on_update)
                )
            out.append(inst)
        insts[:] = out

    def patched(self, ordered):
        for insts in ordered.values():
            _split_block(self.nc, insts)
        return orig_lower(self, ordered)

    tilemod.TileContext._lower_ordered_insts = patched


def _patch_drain(tilemod):
    """walrus in this env rejects >1 sem wait on CTRL instructions: split the
    TileContext-exit drain's waits across single-wait SP NoOPs."""

    def _drain_and_barrier_split(self, tick_clock, wait_clock):
        nc = self.nc
        gc = tick_clock.global_clock
        ticks = [int(t) for t in re.findall(r"\d+", str(gc))]
        for idx, tick in enumerate(ticks):
            if tick <= 0:
                continue
            part = bass_rust.VectorClock()
            part.require_at_least(idx, tick)
            n = nc.sync.nop(hint="drain_split", nofuse=True)
            wait_clock.add_sem_waits(n.ins, ScopedClock({None: part}))
        d = nc.sync.drain()
        wait_clock.add_sem_waits(
            d.ins, ScopedClock({None: gc}), cur_clock=ScopedClock({None: gc})
        )
        nc.all_engine_barrier()
        assert self.sems is not None
        popped = nc._tile_sem_poison_stack.pop()
        assert popped is self._sem_poison
        nc.clear_and_free_semaphores(list(self.sems.allocated().values()))
        nc.all_engine_barrier()

    tilemod.TileContext._drain_and_barrier = _drain_and_barrier_split


_patch_wait_split(tile)
_patch_drain(tile)

_PROGRAM = None


def _build_program():
    nc = bass.Bass()

    xT = nc.declare_dram_parameter("xT", [DIM, S], BF, isOutput=False)
    wq = nc.declare_dram_parameter("wq", [DIM, HLOC * D], BF, isOutput=False)
    wk = nc.declare_dram_parameter("wk", [DIM, GLOC * D], BF, isOutput=False)
    wv = nc.declare_dram_parameter("wv", [DIM, GLOC * D], BF, isOutput=False)
    wo = nc.declare_dram_parameter("wo", [HLOC * D, DIM], BF, isOutput=False)
    cosT = nc.declare_dram_parameter("cosT", [D, S], BF, isOutput=False)
    sinT = nc.declare_dram_parameter("sinT", [D, S], BF, isOutput=False)
    tri = nc.declare_dram_parameter("tri", [D, D], F32, isOutput=False)
    outT = nc.declare_dram_parameter("outT", [DIM, S], BF, isOutput=True)

    # rope pair-swap (partition p <-> p^1) as a 32-lane DVE stream shuffle
    swap_mask = [i ^ 1 for i in range(32)]

    xT_r = xT.rearrange("(o p) t -> p o t", p=128)      # [128, 16, 1024]
    wq_r = wq.rearrange("(o p) f -> p o f", p=128)      # [128, 16, 1024]
    wk_r = wk.rearrange("(o p) f -> p o f", p=128)      # [128, 16, 256]
    wv_r = wv.rearrange("(o p) f -> p o f", p=128)      # [128, 16, 256]
    wo_r = wo.rearrange("(c p) e -> p c e", p=128)      # [128, 8, 2048]
    outT_r = outT.rearrange("(o p) t -> p o t", p=128)  # [128, 16, 1024]

    with tile.TileContext(nc) as tc:
        with (
            tc.tile_pool(name="consts", bufs=1) as consts,
            tc.tile_pool(name="rope", bufs=2) as rope_pool,
            tc.tile_pool(name="dinv", bufs=2) as dinv_pool,
            tc.tile_pool(name="ostage", bufs=3) as ostage,
            tc.tile_pool(name="psA", bufs=2, space="PSUM") as psA,
            tc.tile_pool(name="psS", bufs=3, space="PSUM") as psS,
            tc.tile_pool(name="psO", bufs=2, space="PSUM") as psO,
            tc.tile_pool(name="psD", bufs=1, space="PSUM") as psD,
        ):
            # ---- constant loads (ordered so K proj starts ~7us in and the
            # chunk-major K matmuls ride the incoming xT stream) ----
            xT_sb = consts.tile([128, KC, S], BF)
            wk_sb = consts.tile([128, KC, GLOC * D], BF)
            wq_sb = consts.tile([128, KC, HLOC * D], BF)
            wv_sb = consts.tile([128, KC, GLOC * D], BF)
            nc.sync.dma_start(out=wk_sb[:, 0, :], in_=wk_r[:, 0, :])
            nc.sync.dma_start(out=xT_sb[:, 0, 0:512], in_=xT_r[:, 0, 0:512])
            nc.sync.dma_start(out=xT_sb[:, 0, 512:1024],
                              in_=xT_r[:, 0, 512:1024])
            nc.sync.dma_start(out=wk_sb[:, 1:4, :], in_=wk_r[:, 1:4, :])
            nc.sync.dma_start(out=xT_sb[:, 1, :], in_=xT_r[:, 1, :])
            nc.sync.dma_start(out=xT_sb[:, 2, :], in_=xT_r[:, 2, :])
            nc.sync.dma_start(out=wv_sb[:, 0:4, :], in_=wv_r[:, 0:4, :])
            nc.sync.dma_start(out=wk_sb[:, 4:8, :], in_=wk_r[:, 4:8, :])
            nc.sync.dma_start(out=wv_sb[:, 4:8, :], in_=wv_r[:, 4:8, :])
            nc.sync.dma_start(out=xT_sb[:, 3, :], in_=xT_r[:, 3, :])
            nc.sync.dma_start(out=xT_sb[:, 4, :], in_=xT_r[:, 4, :])
            nc.sync.dma_start(out=wk_sb[:, 8:12, :], in_=wk_r[:, 8:12, :])
            nc.sync.dma_start(out=wv_sb[:, 8:12, :], in_=wv_r[:, 8:12, :])
            nc.sync.dma_start(out=xT_sb[:, 5, :], in_=xT_r[:, 5, :])
            nc.sync.dma_start(out=xT_sb[:, 6:8, :], in_=xT_r[:, 6:8, :])
            nc.sync.dma_start(out=wk_sb[:, 12:16, :], in_=wk_r[:, 12:16, :])
            nc.sync.dma_start(out=wv_sb[:, 12:16, :], in_=wv_r[:, 12:16, :])
            nc.sync.dma_start(out=xT_sb[:, 8:10, :], in_=xT_r[:, 8:10, :])
            nc.sync.dma_start(out=xT_sb[:, 10:12, :], in_=xT_r[:, 10:12, :])
            nc.sync.dma_start(out=xT_sb[:, 12:14, :], in_=xT_r[:, 12:14, :])
            nc.sync.dma_start(out=xT_sb[:, 14:16, :], in_=xT_r[:, 14:16, :])
            cos_sb = consts.tile([128, S], BF)
            nc.sync.dma_start(out=cos_sb, in_=cosT[:, :])
            sin_sb = consts.tile([128, S], BF)
            nc.sync.dma_start(out=sin_sb, in_=sinT[:, :])
            tri_sb = consts.tile([128, D], F32)
            nc.sync.dma_start(out=tri_sb, in_=tri[:, :])
            nc.sync.dma_start(out=wq_sb[:, 0:8, :], in_=wq_r[:, 0:8, :])
            nc.sync.dma_start(out=wq_sb[:, 8:16, :], in_=wq_r[:, 8:16, :])
            wo_sb = consts.tile([128, HLOC, DIM], BF)
            nc.sync.dma_start(out=wo_sb[:, :, :], in_=wo_r[:, :, :])

            # all-ones stationary operand (fp8 pair layout): the DoubleRow
            # denom matmul writes the key-sum to EVERY output partition
            ones2 = consts.tile([128, 2, 128], FP8)
            nc.vector.memset(ones2, 1.0)

            # warm-up matmuls on the memset tile while the first input DMAs
            # are in flight: releases the HAM clock throttle (K=4/8 ->
            # 8/8) before the real matmuls arrive, at zero DMA cost
            for w in range(10):
                wps = psO.tile([128, 512], F32, tag="psO")
                nc.tensor.matmul(
                    wps[:, 0:256],
                    ones2[:, 0, :],
                    ones2[:, :, :],
                    start=True,
                    stop=True,
                )

            # persistent activations
            qT_sb = consts.tile([128, HLOC, S], BF)   # q^T, rotated
            kT_sb = consts.tile([128, GLOC, S], BF)   # k^T, rotated
            v_sb = consts.tile([128, NKB, GLOC * D], BF)  # v in [tok, d]
            aT_sb = consts.tile([128, HLOC, S], BF)   # attention out^T

            # persistent p^T tiles (fp8), two-deep per row-group so head h+1's
            # exp can overlap head h's attnV.  The never-exp-written "gap"
            # columns of odd diagonal blocks are zeroed once and stay 0, so
            # the paired DoubleRow denom matmuls read 0 there.
            pt0 = [
                consts.tile([128, 4, 512], FP8, name=f"pt0_{i}")
                for i in range(2)
            ]
            pt1 = [
                consts.tile([128, NKB, 512], FP8, name=f"pt1_{i}")
                for i in range(2)
            ]
            for t in pt0:
                nc.vector.memset(t[:, 1, 0:128], 0.0)
                nc.vector.memset(t[:, 3, 256:384], 0.0)
            for t in pt1:
                nc.vector.memset(t[:, 5, 0:128], 0.0)
                nc.vector.memset(t[:, 7, 256:384], 0.0)

            def rope_combine(ps, tb, dst_sb, dst_idx):
                """dst[:, dst_idx, tb*512:+512] = rope(ps) via DVE shuffle."""
                qsb = rope_pool.tile([128, 512], BF, tag="qsb")
                nc.scalar.copy(out=qsb, in_=ps)
                qsw = rope_pool.tile([128, 512], BF, tag="qsw")
                nc.vector.stream_shuffle(out=qsw, in_=qsb, mask=swap_mask)
                tspan = slice(tb * 512, (tb + 1) * 512)
                t1 = rope_pool.tile([128, 512], BF, tag="t1")
                nc.vector.tensor_mul(out=t1, in0=qsb, in1=cos_sb[:, tspan])
                t2 = rope_pool.tile([128, 512], BF, tag="t2")
                nc.vector.tensor_mul(out=t2, in0=qsw, in1=sin_sb[:, tspan])
                nc.vector.tensor_add(
                    out=dst_sb[:, dst_idx, tspan], in0=t1, in1=t2
                )

            def proj_rope(w_sb, hd_idx, tb, dst_sb, dst_idx):
                """dst[:, dst_idx, tb*512:+512] = rope(w^T @ x^T) tile."""
                ps = psA.tile([128, 512], F32, tag="ps")
                for kc in range(KC):
                    nc.tensor.matmul(
                        ps,
                        w_sb[:, kc, hd_idx * 128:(hd_idx + 1) * 128],
                        xT_sb[:, kc, tb * 512:(tb + 1) * 512],
                        start=(kc == 0),
                        stop=(kc == KC - 1),
                    )
                rope_combine(ps, tb, dst_sb, dst_idx)

            # ---- K + V projections, chunk-major so the accumulations ride
            # the incoming xT chunk stream instead of head-of-line blocking.
            # V packs two token blocks per [128,512] PSUM tile. ----
            kacc = []
            for g in range(GLOC):
                for tb in range(NTB):
                    pool = psA if g == 0 else psO
                    kacc.append(pool.tile([128, 512], F32,
                                          tag="ps" if g == 0 else "psO",
                                          name=f"kacc{g}{tb}"))
            # v accumulators for token blocks 0-3 ride the xT stream too.
            # One PSUM tile per block: a matmul's start=True clears the WHOLE
            # bank's has_written bits, so two interleaved accumulation groups
            # must never share a bank.
            vacc = [
                psS.tile([128, 512], F32, tag="psS", name=f"vacc{i}")
                for i in range(3)
            ] + [psD.tile([128, 512], F32, tag="psD", name="vacc3")]
            # per chunk: tb=0 K matmuls and v-blocks 0-3 only touch the first
            # 512 columns, so they run while the chunk's second half streams
            kidx = {(g, tb): i for i, (g, tb) in enumerate(
                (g, tb) for g in range(GLOC) for tb in range(NTB)
            )}
            # V runs two chunk-groups behind K (accumulation order is free)
            # so the wv stream can load after xT2 instead of delaying it
            for grp in range(KC + 2):
                kc = grp
                vc = grp - 2
                if kc < KC:
                    for g, tb in ((0, 0), (1, 0)):
                        nc.tensor.matmul(
                            kacc[kidx[(g, tb)]],
                            wk_sb[:, kc, g * 128:(g + 1) * 128],
                            xT_sb[:, kc, tb * 512:(tb + 1) * 512],
                            start=(kc == 0),
                            stop=(kc == KC - 1),
                        )
                if 0 <= vc < KC:
                    for vtb in range(4):
                        nc.tensor.matmul(
                            vacc[vtb][:, : GLOC * D],
                            xT_sb[:, vc, vtb * 128:(vtb + 1) * 128],
                            wv_sb[:, vc, :],
                            start=(vc == 0),
                            stop=(vc == KC - 1),
                        )
                if kc < KC:
                    for g, tb in ((0, 1), (1, 1)):
                        nc.tensor.matmul(
                            kacc[kidx[(g, tb)]],
                            wk_sb[:, kc, g * 128:(g + 1) * 128],
                            xT_sb[:, kc, tb * 512:(tb + 1) * 512],
                            start=(kc == 0),
                            stop=(kc == KC - 1),
                        )
            for i, (g, tb) in enumerate(
                (g, tb) for g in range(GLOC) for tb in range(NTB)
            ):
                rope_combine(kacc[i], tb, kT_sb, g)
            for vtb in range(4):
                nc.scalar.copy(out=v_sb[:, vtb, :], in_=vacc[vtb][:, : GLOC * D])
            for vtb in range(4, NKB):
                ps = psA.tile([128, 512], F32, tag="ps")
                for kc in range(KC):
                    nc.tensor.matmul(
                        ps[:, : GLOC * D],
                        xT_sb[:, kc, vtb * 128:(vtb + 1) * 128],
                        wv_sb[:, kc, :],
                        start=(kc == 0),
                        stop=(kc == KC - 1),
                    )
                nc.scalar.copy(out=v_sb[:, vtb, :], in_=ps[:, : GLOC * D])

            def score_block(h, rg, kb, pt):
                """scores matmul + mask + exp -> pt for one 128-key block."""
                g = h // REP
                c0 = max(0, kb - 4 * rg)
                span = slice(c0 * 128, 512)
                ps = psS.tile([128, 512], F32, tag="psS")
                nc.tensor.matmul(
                    ps[:, span],
                    kT_sb[:, g, kb * 128:(kb + 1) * 128],
                    qT_sb[:, h, rg * 512 + c0 * 128:(rg + 1) * 512],
                    start=True,
                    stop=True,
                )
                if kb - 4 * rg >= 0:
                    cc = kb - 4 * rg
                    nc.vector.tensor_add(
                        out=ps[:, cc * 128:(cc + 1) * 128],
                        in0=ps[:, cc * 128:(cc + 1) * 128],
                        in1=tri_sb,
                    )
                nc.scalar.activation(
                    out=pt[:, kb, span],
                    in_=ps[:, span],
                    func=mybir.ActivationFunctionType.Exp,
                    scale=SCALE,
                )

            def consume_block(h, rg, kb, pt, po, pd):
                """attn@V for block kb (+ paired DoubleRow denom on odd kb)."""
                g = h // REP
                nkb = 4 * rg + 4
                c0 = max(0, kb - 4 * rg)
                span = slice(c0 * 128, 512)
                # attn@V per block: bf16 v stationary, fp8 p^T moving
                nc.tensor.matmul(
                    po[:, span],
                    v_sb[:, kb, g * D:(g + 1) * D],
                    pt[:, kb, span],
                    start=(kb == 0),
                    stop=(kb == nkb - 1),
                )
                if kb % 2 == 1:
                    p0 = kb - 1
                    c00 = max(0, p0 - 4 * rg)
                    spanp = slice(c00 * 128, 512)
                    nc.tensor.matmul(
                        pd[:, spanp],
                        ones2,
                        pt[:, p0:p0 + 2, spanp],
                        start=(p0 == 0),
                        stop=(p0 == nkb - 2),
                        perf_mode=DR,
                    )

            def normalize(h, rg, po, pd):
                # 1/denom as exp(-ln(denom)) on ACT: DVE reciprocal costs
                # ~6.5ns/elem and the fast-approx custom op doesn't
                # compile on this walrus ("ISA wrong length")
                rows = slice(rg * 512, (rg + 1) * 512)
                lnd = dinv_pool.tile([128, 512], F32, tag="lnd")
                nc.scalar.activation(
                    out=lnd, in_=pd, func=mybir.ActivationFunctionType.Ln
                )
                dinv_b = dinv_pool.tile([128, 512], F32, tag="dinvb")
                nc.scalar.activation(
                    out=dinv_b, in_=lnd,
                    func=mybir.ActivationFunctionType.Exp, scale=-1.0,
                )
                nc.vector.tensor_mul(
                    out=aT_sb[:, h, rows],
                    in0=po,
                    in1=dinv_b,
                )

            def attention_rg(h, rg):
                pt = (pt0 if rg == 0 else pt1)[h % 2]
                po = psO.tile([128, 512], F32, tag="psO")
                pd = psD.tile([128, 512], F32, tag="psD")
                for kb in range(4 * rg + 4):
                    score_block(h, rg, kb, pt)
                    consume_block(h, rg, kb, pt, po, pd)
                normalize(h, rg, po, pd)

            def op_group(et, tb):
                """one [128, 512] chunk of out^T = wo_sh^T @ a^T."""
                ps = psA.tile([128, 512], F32, tag="ps")
                for c in range(HLOC):
                    nc.tensor.matmul(
                        ps,
                        wo_sb[:, c, et * 128:(et + 1) * 128],
                        aT_sb[:, c, tb * 512:(tb + 1) * 512],
                        start=(c == 0),
                        stop=(c == HLOC - 1),
                    )
                st = ostage.tile([128, 512], BF, tag="st")
                nc.vector.tensor_copy(out=st, in_=ps)
                nc.sync.dma_start(
                    out=outT_r[:, et, tb * 512:(tb + 1) * 512], in_=st
                )

            # ---- rg0 sweep: project q for head h+1, attend head h rows
            # 0-511 (q-proj matmuls absorb the ACT exp latency) ----
            for tb in range(NTB):
                proj_rope(wq_sb, 0, tb, qT_sb, 0)
            for h in range(HLOC):
                if h + 1 < HLOC:
                    for tb in range(NTB):
                        proj_rope(wq_sb, h + 1, tb, qT_sb, h + 1)
                attention_rg(h, 0)

            # ---- rg1 sweep woven with tb=0 out-proj chunks (all heads'
            # rows 0-511 of a^T are done, so those 16 chunks keep the PE
            # busy while ACT works through the rg1 exps) ----
            for h in range(HLOC):
                if h > 0:
                    op_group(2 * h - 2, 0)
                    op_group(2 * h - 1, 0)
                attention_rg(h, 1)
            # the trailing tb=0 chunks absorb head 7's ACT/DVE drain before
            # the tb=1 out-proj sweep begins
            op_group(KC - 2, 0)
            op_group(KC - 1, 0)
            for et in range(KC - 1):
                op_group(et, 1)
            # final chunk as two half-width accumulation groups (separate
            # PSUM buffers!) so the first half's staging copy + DMA overlap
            # the second half's matmuls instead of trailing them
            for half in range(2):
                ph = psA.tile([128, 512], F32, tag="ps", name=f"ps_l{half}")
                cols = slice(512 + half * 256, 512 + (half + 1) * 256)
                for c in range(HLOC):
                    nc.tensor.matmul(
                        ph[:, 0:256],
                        wo_sb[:, c, (KC - 1) * 128:KC * 128],
                        aT_sb[:, c, cols],
                        start=(c == 0),
                        stop=(c == HLOC - 1),
                    )
                sth = ostage.tile([128, 256], BF, tag="sth", name=f"sth{half}")
                nc.vector.tensor_copy(out=sth, in_=ph[:, 0:256])
                nc.sync.dma_start(out=outT_r[:, KC - 1, cols], in_=sth)

    return nc


LAST_RESULT = None
_TRACE = os.environ.get("BASS_ATTN_TRACE", "") == "1"

if _TRACE:
    # Register the NTFF profile hook that the agent image's antenv lacks
    # (test/profiling only; the graded path never enters this branch).
    try:
        import sys
        import types

        import antenv  # noqa: F401

        if "antenv.axon_hooks" not in sys.modules:
            _mod = types.ModuleType("antenv.axon_hooks")
            _hook_box = [None]
            _mod.set_axon_ntff_profile_hook = lambda h: _hook_box.__setitem__(0, h)
            _mod.get_axon_ntff_profile_hook = lambda: _hook_box[0]
            sys.modules["antenv.axon_hooks"] = _mod
            import antenv as _antenv

            _antenv.axon_hooks = _mod
            from trn_agent_boot.trn_boot import _ntff_profile_via_ctypes

            _mod.set_axon_ntff_profile_hook(
                _ntff_profile_via_ctypes("/opt/axon/libaxon_pjrt.so")
            )
    except Exception as e:  # pragma: no cover
        print(f"NTFF hook setup failed ({e}); tracing will be skipped")


def kernel(x, freqs_cis, wq, wk, wv, wo, seq_len=None, **_ignored):
    global _PROGRAM, LAST_RESULT
    x = np.ascontiguousarray(np.asarray(x, dtype=np.float32))
    fc = np.asarray(freqs_cis, dtype=np.float32)
    wq = np.asarray(wq, dtype=np.float32)
    wk = np.asarray(wk, dtype=np.float32)
    wv = np.asarray(wv, dtype=np.float32)
    wo = np.asarray(wo, dtype=np.float32)

    # host-side prep (sharding + transposed/bf16 views + rope/mask constants)
    xT = np.ascontiguousarray(x.T).astype(BF16)                    # [2048, 4096]
    cos = np.ascontiguousarray(np.repeat(fc[:S, :, 0], 2, axis=1).T).astype(BF16)
    sgn = np.where(np.arange(D) % 2 == 0, -1.0, 1.0).astype(np.float32)
    sin = np.ascontiguousarray(
        (np.repeat(fc[:S, :, 1], 2, axis=1) * sgn[None, :]).T
    ).astype(BF16)
    k_idx = np.arange(128)[:, None]
    r_idx = np.arange(128)[None, :]
    tri = np.where(r_idx >= k_idx, 0.0, -1e9).astype(np.float32)

    in_maps = []
    for c in range(NCORE):
        s, h2 = c // 2, c % 2
        in_maps.append(
            {
                "xT": np.ascontiguousarray(xT[:, s * S:(s + 1) * S]),
                "wq": wq[:, h2 * HLOC * D:(h2 + 1) * HLOC * D].astype(BF16),
                "wk": wk[:, h2 * GLOC * D:(h2 + 1) * GLOC * D].astype(BF16),
                "wv": wv[:, h2 * GLOC * D:(h2 + 1) * GLOC * D].astype(BF16),
                "wo": wo[h2 * HLOC * D:(h2 + 1) * HLOC * D, :].astype(BF16),
                "cosT": cos,
                "sinT": sin,
                "tri": tri,
            }
        )

    if _PROGRAM is None:
        _PROGRAM = _build_program()

    res = run_bass_kernel_spmd(
        _PROGRAM, in_maps, core_ids=list(range(NCORE)), trace=_TRACE
    )
    LAST_RESULT = res

    out = np.empty((B * S, DIM), np.float32)
    for s in range(B):
        outT = (
            res.results[2 * s]["outT"].astype(np.float32)
            + res.results[2 * s + 1]["outT"].astype(np.float32)
        )
        out[s * S:(s + 1) * S, :] = outT.T
    return out


# revision 43
# speedup vs baseline: 1.0610x; 1.0007x over previous
"""Trainium2 Bass kernel for packed-segment causal GQA attention.

Shapes (hardcoded): x [4096, 2048], 16 q heads / 4 kv heads, head dim 128,
4 packed segments of 1024 tokens.

Sharding over 8 cores: core c -> segment c//2 (tokens), head-half c%2
(8 q heads + their 2 kv heads; wq/wk/wv column-sharded, wo row-sharded).
Each core computes a partial out^T [2048, 1024] for its segment; host sums
the two partials per segment (wo row-shard) and transposes back.

Optimizations vs the 244us bf16 baseline (now ~205us, PE ~90% busy):
- p^T = exp(s) stored fp8e4 (the only tensor whose quantization noise fits
  the error budget - weight-side fp8 noise does NOT average out through
  attention; measured ablation puts any weight-side fp8 at 1.5-3.9% relmax
  vs the 2% gate).  The softmax-denominator matmul then runs as fp8
  DoubleRow over PAIRED key blocks (half the PE streaming) while attn@V
  consumes the same fp8 p^T against a bf16 stationary v (mixed-dtype
  matmul).  Projections/scores/attn@V stay bf16.
- K and the first 4 V token blocks are projected chunk-major, riding the
  incoming xT DMA stream (per-chunk loads interleaved wk/wv/xT).  One
  PSUM bank per accumulation group: matmul start=True clears the whole
  bank's has_written bits, so groups must never share a bank.
- RoPE pair-swap runs as a DVE stream_shuffle (mask p^1 within 32-lane
  groups) instead of a PE permutation matmul; rope muls/adds are bf16.
- Head loop split into an rg0 sweep (rows 0-511, q-proj of head h+1
  woven in) and an rg1 sweep woven with the tb=0 out-projection chunks,
  so the PE never waits on ACT exps; warm-up matmuls on the memset ones
  tile release the HAM clock throttle during the first input DMAs.
- Output DMA'd bf16 (halved), summed across the wo row-shard pair in
  fp32 on host.  1/denom via exp(-ln(d)) on ACT (DVE reciprocal too slow,
  custom fast-approx DVE op doesn't compile on this walrus).

On-device dataflow (all in transposed token space, fp32 PSUM):
  q^T = wq_sh^T @ x_seg^T            (lhsT = wq_sh, rhs = x^T, bf16)
  RoPE via pair-swap matmul + cos/sin elementwise combine (bf16)
  s^T[key, row] = k^T_tile^T.T @ q^T  per 128-key x 512-row block
  p^T = exp(s^T/sqrt(d)) -> fp8, additive triangular mask on diag chunks
  denom = ones^T @ p^T  (fp8 DoubleRow over paired key blocks)
  o^T += v_tile.T @ p^T (v bf16 stationary, p^T fp8 moving)
  a^T = o^T * exp(-ln(denom));  out^T = wo_sh^T @ a^T (bf16)
"""

import os
import re

import numpy as np
import ml_dtypes

import bass_rust
import concourse.bass as bass
import concourse.mybir as mybir
import concourse.tile as tile
from concourse.bass_utils import run_bass_kernel_spmd
from concourse.vector_clock import ScopedClock

BF16 = ml_dtypes.bfloat16
F32 = mybir.dt.float32
BF = mybir.dt.bfloat16
FP8 = mybir.dt.float8e4
DR = mybir.MatmulPerfMode.DoubleRow

DIM, H, HKV, D, B, S = 2048, 16, 4, 128, 4, 1024
REP = H // HKV
SCALE = float(D) ** -0.5
NCORE = 8
HLOC = 8          # q heads per core
GLOC = 2          # kv heads per core
KC = DIM // 128   # 16 contraction chunks
NTB = S // 512    # 2 row blocks of 512
NKB = S // 128    # 8 key blocks of 128


_MAXW = 1


def _patch_wait_split(tilemod):
    """walrus in this env caps sem waits per instruction: rewrite any
    instruction carrying more than _MAXW waits so the excess waits land on
    same-engine NoOps inserted just before it."""

    orig_lower = tilemod.TileContext._lower_ordered_insts

    def _split_block(nc, insts):
        out = []
        for inst in insts:
            si = getattr(inst, "sync_info", None)
            waits = list(si.on_wait) if si is not None and si.on_wait else []
            if len(waits) > _MAXW:
                head, rest = waits[:-_MAXW], waits[-_MAXW:]
                for w in head:  # NoOp is CTRL-class: keep it to 1 wait each
                    out.append(
                        mybir.InstNoOp(
                            name=nc.get_next_instruction_name(),
                            engine=inst.engine,
                            bass_nofuse=True,
                            debug=inst.debug,
                            sync_info=mybir.SyncInfo(on_wait=[w], on_update=[]),
                        )
                    )
                inst.sync_info = mybir.SyncInfo(
                    on_wait=rest, on_update=list(si.on_update)
                )
            out.append(inst)
        insts[:] = out

    def patched(self, ordered):
        for insts in ordered.values():
            _split_block(self.nc, insts)
        return orig_lower(self, ordered)

    tilemod.TileContext._lower_ordered_insts = patched


def _patch_drain(tilemod):
    """walrus in this env rejects >1 sem wait on CTRL instructions: split the
    TileContext-exit drain's waits across single-wait SP NoOPs."""

    def _drain_and_barrier_split(self, tick_clock, wait_clock):
        nc = self.nc
        gc = tick_clock.global_clock
        ticks = [int(t) for t in re.findall(r"\d+", str(gc))]
        for idx, tick in enumerate(ticks):
            if tick <= 0:
                continue
            part = bass_rust.VectorClock()
            part.require_at_least(idx, tick)
            n = nc.sync.nop(hint="drain_split", nofuse=True)
            wait_clock.add_sem_waits(n.ins, ScopedClock({None: part}))
        d = nc.sync.drain()
        wait_clock.add_sem_waits(
            d.ins, ScopedClock({None: gc}), cur_clock=ScopedClock({None: gc})
        )
        nc.all_engine_barrier()
        assert self.sems is not None
        popped = nc._tile_sem_poison_stack.pop()
        assert popped is self._sem_poison
        nc.clear_and_free_semaphores(list(self.sems.allocated().values()))
        nc.all_engine_barrier()

    tilemod.TileContext._drain_and_barrier = _drain_and_barrier_split


_patch_wait_split(tile)
_patch_drain(tile)

_PROGRAM = None


def _build_program():
    nc = bass.Bass()

    xT = nc.declare_dram_parameter("xT", [DIM, S], BF, isOutput=False)
    wq = nc.declare_dram_parameter("wq", [DIM, HLOC * D], BF, isOutput=False)
    wk = nc.declare_dram_parameter("wk", [DIM, GLOC * D], BF, isOutput=False)
    wv = nc.declare_dram_parameter("wv", [DIM, GLOC * D], BF, isOutput=False)
    wo = nc.declare_dram_parameter("wo", [HLOC * D, DIM], BF, isOutput=False)
    cosT = nc.declare_dram_parameter("cosT", [D, S], BF, isOutput=False)
    sinT = nc.declare_dram_parameter("sinT", [D, S], BF, isOutput=False)
    tri = nc.declare_dram_parameter("tri", [D, D], F32, isOutput=False)
    outT = nc.declare_dram_parameter("outT", [DIM, S], BF, isOutput=True)

    # rope pair-swap (partition p <-> p^1) as a 32-lane DVE stream shuffle
    swap_mask = [i ^ 1 for i in range(32)]

    xT_r = xT.rearrange("(o p) t -> p o t", p=128)      # [128, 16, 1024]
    wq_r = wq.rearrange("(o p) f -> p o f", p=128)      # [128, 16, 1024]
    wk_r = wk.rearrange("(o p) f -> p o f", p=128)      # [128, 16, 256]
    wv_r = wv.rearrange("(o p) f -> p o f", p=128)      # [128, 16, 256]
    wo_r = wo.rearrange("(c p) e -> p c e", p=128)      # [128, 8, 2048]
    outT_r = outT.rearrange("(o p) t -> p o t", p=128)  # [128, 16, 1024]

    with tile.TileContext(nc) as tc:
        with (
            tc.tile_pool(name="consts", bufs=1) as consts,
            tc.tile_pool(name="rope", bufs=2) as rope_pool,
            tc.tile_pool(name="dinv", bufs=2) as dinv_pool,
            tc.tile_pool(name="ostage", bufs=3) as ostage,
            tc.tile_pool(name="psA", bufs=2, space="PSUM") as psA,
            tc.tile_pool(name="psS", bufs=3, space="PSUM") as psS,
            tc.tile_pool(name="psO", bufs=2, space="PSUM") as psO,
            tc.tile_pool(name="psD", bufs=1, space="PSUM") as psD,
        ):
            # ---- constant loads (ordered so K proj starts ~7us in and the
            # chunk-major K matmuls ride the incoming xT stream) ----
            xT_sb = consts.tile([128, KC, S], BF)
            wk_sb = consts.tile([128, KC, GLOC * D], BF)
            wq_sb = consts.tile([128, KC, HLOC * D], BF)
            wv_sb = consts.tile([128, KC, GLOC * D], BF)
            nc.sync.dma_start(out=wk_sb[:, 0, :], in_=wk_r[:, 0, :])
            nc.sync.dma_start(out=xT_sb[:, 0, 0:512], in_=xT_r[:, 0, 0:512])
            nc.sync.dma_start(out=xT_sb[:, 0, 512:1024],
                              in_=xT_r[:, 0, 512:1024])
            nc.sync.dma_start(out=wk_sb[:, 1:4, :], in_=wk_r[:, 1:4, :])
            nc.sync.dma_start(out=xT_sb[:, 1, :], in_=xT_r[:, 1, :])
            nc.sync.dma_start(out=xT_sb[:, 2, :], in_=xT_r[:, 2, :])
            nc.sync.dma_start(out=wv_sb[:, 0:4, :], in_=wv_r[:, 0:4, :])
            nc.sync.dma_start(out=wk_sb[:, 4:8, :], in_=wk_r[:, 4:8, :])
            nc.sync.dma_start(out=wv_sb[:, 4:8, :], in_=wv_r[:, 4:8, :])
            nc.sync.dma_start(out=xT_sb[:, 3, :], in_=xT_r[:, 3, :])
            nc.sync.dma_start(out=xT_sb[:, 4, :], in_=xT_r[:, 4, :])
            nc.sync.dma_start(out=wk_sb[:, 8:12, :], in_=wk_r[:, 8:12, :])
            nc.sync.dma_start(out=wv_sb[:, 8:12, :], in_=wv_r[:, 8:12, :])
            nc.sync.dma_start(out=xT_sb[:, 5, :], in_=xT_r[:, 5, :])
            nc.sync.dma_start(out=xT_sb[:, 6:8, :], in_=xT_r[:, 6:8, :])
            nc.sync.dma_start(out=wk_sb[:, 12:16, :], in_=wk_r[:, 12:16, :])
            nc.sync.dma_start(out=wv_sb[:, 12:16, :], in_=wv_r[:, 12:16, :])
            nc.sync.dma_start(out=xT_sb[:, 8:10, :], in_=xT_r[:, 8:10, :])
            nc.sync.dma_start(out=xT_sb[:, 10:12, :], in_=xT_r[:, 10:12, :])
            nc.sync.dma_start(out=xT_sb[:, 12:14, :], in_=xT_r[:, 12:14, :])
            nc.sync.dma_start(out=xT_sb[:, 14:16, :], in_=xT_r[:, 14:16, :])
            cos_sb = consts.tile([128, S], BF)
            nc.sync.dma_start(out=cos_sb, in_=cosT[:, :])
            sin_sb = consts.tile([128, S], BF)
            nc.sync.dma_start(out=sin_sb, in_=sinT[:, :])
            tri_sb = consts.tile([128, D], F32)
            nc.sync.dma_start(out=tri_sb, in_=tri[:, :])
            nc.sync.dma_start(out=wq_sb[:, 0:8, :], in_=wq_r[:, 0:8, :])
            nc.sync.dma_start(out=wq_sb[:, 8:16, :], in_=wq_r[:, 8:16, :])
            wo_sb = consts.tile([128, HLOC, DIM], BF)
            nc.sync.dma_start(out=wo_sb[:, :, :], in_=wo_r[:, :, :])

            # all-ones stationary operand (fp8 pair layout): the DoubleRow
            # denom matmul writes the key-sum to EVERY output partition
            ones2 = consts.tile([128, 2, 128], FP8)
            nc.vector.memset(ones2, 1.0)

            # warm-up matmuls on the memset tile while the first input DMAs
            # are in flight: releases the HAM clock throttle (K=4/8 ->
            # 8/8) before the real matmuls arrive, at zero DMA cost
            for w in range(10):
                wps = psO.tile([128, 512], F32, tag="psO")
                nc.tensor.matmul(
                    wps[:, 0:256],
                    ones2[:, 0, :],
                    ones2[:, :, :],
                    start=True,
                    stop=True,
                )

            # persistent activations
            qT_sb = consts.tile([128, HLOC, S], BF)   # q^T, rotated
            kT_sb = consts.tile([128, GLOC, S], BF)   # k^T, rotated
            v_sb = consts.tile([128, NKB, GLOC * D], BF)  # v in [tok, d]
            aT_sb = consts.tile([128, HLOC, S], BF)   # attention out^T

            # persistent p^T tiles (fp8), two-deep per row-group so head h+1's
            # exp can overlap head h's attnV.  The never-exp-written "gap"
            # columns of odd diagonal blocks are zeroed once and stay 0, so
            # the paired DoubleRow denom matmuls read 0 there.
            pt0 = [
                consts.tile([128, 4, 512], FP8, name=f"pt0_{i}")
                for i in range(2)
            ]
            pt1 = [
                consts.tile([128, NKB, 512], FP8, name=f"pt1_{i}")
                for i in range(2)
            ]
            for t in pt0:
                nc.vector.memset(t[:, 1, 0:128], 0.0)
                nc.vector.memset(t[:, 3, 256:384], 0.0)
            for t in pt1:
                nc.vector.memset(t[:, 5, 0:128], 0.0)
                nc.vector.memset(t[:, 7, 256:384], 0.0)

            def rope_combine(ps, tb, dst_sb, dst_idx):
                """dst[:, dst_idx, tb*512:+512] = rope(ps) via DVE shuffle."""
                qsb = rope_pool.tile([128, 512], BF, tag="qsb")
                nc.scalar.copy(out=qsb, in_=ps)
                qsw = rope_pool.tile([128, 512], BF, tag="qsw")
                nc.vector.stream_shuffle(out=qsw, in_=qsb, mask=swap_mask)
                tspan = slice(tb * 512, (tb + 1) * 512)
                t1 = rope_pool.tile([128, 512], BF, tag="t1")
                nc.vector.tensor_mul(out=t1, in0=qsb, in1=cos_sb[:, tspan])
                t2 = rope_pool.tile([128, 512], BF, tag="t2")
                nc.vector.tensor_mul(out=t2, in0=qsw, in1=sin_sb[:, tspan])
                nc.vector.tensor_add(
                    out=dst_sb[:, dst_idx, tspan], in0=t1, in1=t2
                )

            def proj_rope(w_sb, hd_idx, tb, dst_sb, dst_idx):
                """dst[:, dst_idx, tb*512:+512] = rope(w^T @ x^T) tile."""
                ps = psA.tile([128, 512], F32, tag="ps")
                for kc in range(KC):
                    nc.tensor.matmul(
                        ps,
                        w_sb[:, kc, hd_idx * 128:(hd_idx + 1) * 128],
                        xT_sb[:, kc, tb * 512:(tb + 1) * 512],
                        start=(kc == 0),
                        stop=(kc == KC - 1),
                    )
                rope_combine(ps, tb, dst_sb, dst_idx)

            # ---- K + V projections, chunk-major so the accumulations ride
            # the incoming xT chunk stream instead of head-of-line blocking.
            # V packs two token blocks per [128,512] PSUM tile. ----
            kacc = []
            for g in range(GLOC):
                for tb in range(NTB):
                    pool = psA if g == 0 else psO
                    kacc.append(pool.tile([128, 512], F32,
                                          tag="ps" if g == 0 else "psO",
                                          name=f"kacc{g}{tb}"))
            # v accumulators for token blocks 0-3 ride the xT stream too.
            # One PSUM tile per block: a matmul's start=True clears the WHOLE
            # bank's has_written bits, so two interleaved accumulation groups
            # must never share a bank.
            vacc = [
                psS.tile([128, 512], F32, tag="psS", name=f"vacc{i}")
                for i in range(3)
            ] + [psD.tile([128, 512], F32, tag="psD", name="vacc3")]
            # per chunk: tb=0 K matmuls and v-blocks 0-3 only touch the first
            # 512 columns, so they run while the chunk's second half streams
            kidx = {(g, tb): i for i, (g, tb) in enumerate(
                (g, tb) for g in range(GLOC) for tb in range(NTB)
            )}
            # V runs two chunk-groups behind K (accumulation order is free)
            # so the wv stream can load after xT2 instead of delaying it
            for grp in range(KC + 2):
                kc = grp
                vc = grp - 2
                if kc < KC:
                    for g, tb in ((0, 0), (1, 0)):
                        nc.tensor.matmul(
                            kacc[kidx[(g, tb)]],
                            wk_sb[:, kc, g * 128:(g + 1) * 128],
                            xT_sb[:, kc, tb * 512:(tb + 1) * 512],
                            start=(kc == 0),
                            stop=(kc == KC - 1),
                        )
                if 0 <= vc < KC:
                    for vtb in range(4):
                        nc.tensor.matmul(
                            vacc[vtb][:, : GLOC * D],
                            xT_sb[:, vc, vtb * 128:(vtb + 1) * 128],
                            wv_sb[:, vc, :],
                            start=(vc == 0),
                            stop=(vc == KC - 1),
                        )
                if kc < KC:
                    for g, tb in ((0, 1), (1, 1)):
                        nc.tensor.matmul(
                            kacc[kidx[(g, tb)]],
                            wk_sb[:, kc, g * 128:(g + 1) * 128],
                            xT_sb[:, kc, tb * 512:(tb + 1) * 512],
                            start=(kc == 0),
                            stop=(kc == KC - 1),
                        )
            for i, (g, tb) in enumerate(
                (g, tb) for g in range(GLOC) for tb in range(NTB)
            ):
                rope_combine(kacc[i], tb, kT_sb, g)
            for vtb in range(4):
                nc.scalar.copy(out=v_sb[:, vtb, :], in_=vacc[vtb][:, : GLOC * D])
            for vtb in range(4, NKB):
                ps = psA.tile([128, 512], F32, tag="ps")
                for kc in range(KC):
                    nc.tensor.matmul(
                        ps[:, : GLOC * D],
                        xT_sb[:, kc, vtb * 128:(vtb + 1) * 128],
                        wv_sb[:, kc, :],
                        start=(kc == 0),
                        stop=(kc == KC - 1),
                    )
                nc.scalar.copy(out=v_sb[:, vtb, :], in_=ps[:, : GLOC * D])

            def score_block(h, rg, kb, pt):
                """scores matmul + mask + exp -> pt for one 128-key block."""
                g = h // REP
                c0 = max(0, kb - 4 * rg)
                span = slice(c0 * 128, 512)
                ps = psS.tile([128, 512], F32, tag="psS")
                nc.tensor.matmul(
                    ps[:, span],
                    kT_sb[:, g, kb * 128:(kb + 1) * 128],
                    qT_sb[:, h, rg * 512 + c0 * 128:(rg + 1) * 512],
                    start=True,
                    stop=True,
                )
                if kb - 4 * rg >= 0:
                    cc = kb - 4 * rg
                    nc.vector.tensor_add(
                        out=ps[:, cc * 128:(cc + 1) * 128],
                        in0=ps[:, cc * 128:(cc + 1) * 128],
                        in1=tri_sb,
                    )
                nc.scalar.activation(
                    out=pt[:, kb, span],
                    in_=ps[:, span],
                    func=mybir.ActivationFunctionType.Exp,
                    scale=SCALE,
                )

            def consume_block(h, rg, kb, pt, po, pd):
                """attn@V for block kb (+ paired DoubleRow denom on odd kb)."""
                g = h // REP
                nkb = 4 * rg + 4
                c0 = max(0, kb - 4 * rg)
                span = slice(c0 * 128, 512)
                # attn@V per block: bf16 v stationary, fp8 p^T moving
                nc.tensor.matmul(
                    po[:, span],
                    v_sb[:, kb, g * D:(g + 1) * D],
                    pt[:, kb, span],
                    start=(kb == 0),
                    stop=(kb == nkb - 1),
                )
                if kb % 2 == 1:
                    p0 = kb - 1
                    c00 = max(0, p0 - 4 * rg)
                    spanp = slice(c00 * 128, 512)
                    nc.tensor.matmul(
                        pd[:, spanp],
                        ones2,
                        pt[:, p0:p0 + 2, spanp],
                        start=(p0 == 0),
                        stop=(p0 == nkb - 2),
                        perf_mode=DR,
                    )

            def normalize(h, rg, po, pd):
                # 1/denom as exp(-ln(denom)) on ACT: DVE reciprocal costs
                # ~6.5ns/elem and the fast-approx custom op doesn't
                # compile on this walrus ("ISA wrong length")
                rows = slice(rg * 512, (rg + 1) * 512)
                lnd = dinv_pool.tile([128, 512], F32, tag="lnd")
                nc.scalar.activation(
                    out=lnd, in_=pd, func=mybir.ActivationFunctionType.Ln
                )
                dinv_b = dinv_pool.tile([128, 512], F32, tag="dinvb")
                nc.scalar.activation(
                    out=dinv_b, in_=lnd,
                    func=mybir.ActivationFunctionType.Exp, scale=-1.0,
                )
                nc.vector.tensor_mul(
                    out=aT_sb[:, h, rows],
                    in0=po,
                    in1=dinv_b,
                )

            def attention_rg(h, rg):
                pt = (pt0 if rg == 0 else pt1)[h % 2]
                po = psO.tile([128, 512], F32, tag="psO")
                pd = psD.tile([128, 512], F32, tag="psD")
                for kb in range(4 * rg + 4):
                    score_block(h, rg, kb, pt)
                    consume_block(h, rg, kb, pt, po, pd)
                normalize(h, rg, po, pd)

            def op_group(et, tb):
                """one [128, 512] chunk of out^T = wo_sh^T @ a^T."""
                ps = psA.tile([128, 512], F32, tag="ps")
                for c in range(HLOC):
                    nc.tensor.matmul(
                        ps,
                        wo_sb[:, c, et * 128:(et + 1) * 128],
                        aT_sb[:, c, tb * 512:(tb + 1) * 512],
                        start=(c == 0),
                        stop=(c == HLOC - 1),
                    )
                st = ostage.tile([128, 512], BF, tag="st")
                nc.vector.tensor_copy(out=st, in_=ps)
                nc.sync.dma_start(
                    out=outT_r[:, et, tb * 512:(tb + 1) * 512], in_=st
                )

            # ---- rg0 sweep: project q for head h+1, attend head h rows
            # 0-511 (q-proj matmuls absorb the ACT exp latency) ----
            for tb in range(NTB):
                proj_rope(wq_sb, 0, tb, qT_sb, 0)
            # attend h BEFORE projecting h+1: keeps head h's exps ahead of
            # the q-proj PSUM copies in the ACT FIFO (the attnV/denom
            # matmuls were stalling ~1.5us on exactly that inversion)
            for h in range(HLOC):
                attention_rg(h, 0)
                if h + 1 < HLOC:
                    for tb in range(NTB):
                        proj_rope(wq_sb, h + 1, tb, qT_sb, h + 1)

            # ---- rg1 sweep woven with tb=0 out-proj chunks (all heads'
            # rows 0-511 of a^T are done, so those 16 chunks keep the PE
            # busy while ACT works through the rg1 exps) ----
            for h in range(HLOC):
                if h > 0:
                    op_group(2 * h - 2, 0)
                    op_group(2 * h - 1, 0)
                attention_rg(h, 1)
            # the trailing tb=0 chunks absorb head 7's ACT/DVE drain before
            # the tb=1 out-proj sweep begins
            op_group(KC - 2, 0)
            op_group(KC - 1, 0)
            for et in range(KC - 1):
                op_group(et, 1)
            # final chunk as two half-width accumulation groups (separate
            # PSUM buffers!) so the first half's staging copy + DMA overlap
            # the second half's matmuls instead of trailing them
            for half in range(2):
                ph = psA.tile([128, 512], F32, tag="ps", name=f"ps_l{half}")
                cols = slice(512 + half * 256, 512 + (half + 1) * 256)
                for c in range(HLOC):
                    nc.tensor.matmul(
                        ph[:, 0:256],
                        wo_sb[:, c, (KC - 1) * 128:KC * 128],
                        aT_sb[:, c, cols],
                        start=(c == 0),
                        stop=(c == HLOC - 1),
                    )
                sth = ostage.tile([128, 256], BF, tag="sth", name=f"sth{half}")
                nc.vector.tensor_copy(out=sth, in_=ph[:, 0:256])
                nc.sync.dma_start(out=outT_r[:, KC - 1, cols], in_=sth)

    return nc


LAST_RESULT = None
_TRACE = os.environ.get("BASS_ATTN_TRACE", "") == "1"

if _TRACE:
    # Register the NTFF profile hook that the agent image's antenv lacks
    # (test/profiling only; the graded path never enters this branch).
    try:
        import sys
        import types

        import antenv  # noqa: F401

        if "antenv.axon_hooks" not in sys.modules:
            _mod = types.ModuleType("antenv.axon_hooks")
            _hook_box = [None]
            _mod.set_axon_ntff_profile_hook = lambda h: _hook_box.__setitem__(0, h)
            _mod.get_axon_ntff_profile_hook = lambda: _hook_box[0]
            sys.modules["antenv.axon_hooks"] = _mod
            import antenv as _antenv

            _antenv.axon_hooks = _mod
            from trn_agent_boot.trn_boot import _ntff_profile_via_ctypes

            _mod.set_axon_ntff_profile_hook(
                _ntff_profile_via_ctypes("/opt/axon/libaxon_pjrt.so")
            )
    except Exception as e:  # pragma: no cover
        print(f"NTFF hook setup failed ({e}); tracing will be skipped")


def kernel(x, freqs_cis, wq, wk, wv, wo, seq_len=None, **_ignored):
    global _PROGRAM, LAST_RESULT
    x = np.ascontiguousarray(np.asarray(x, dtype=np.float32))
    fc = np.asarray(freqs_cis, dtype=np.float32)
    wq = np.asarray(wq, dtype=np.float32)
    wk = np.asarray(wk, dtype=np.float32)
    wv = np.asarray(wv, dtype=np.float32)
    wo = np.asarray(wo, dtype=np.float32)

    # host-side prep (sharding + transposed/bf16 views + rope/mask constants)
    xT = np.ascontiguousarray(x.T).astype(BF16)                    # [2048, 4096]
    cos = np.ascontiguousarray(np.repeat(fc[:S, :, 0], 2, axis=1).T).astype(BF16)
    sgn = np.where(np.arange(D) % 2 == 0, -1.0, 1.0).astype(np.float32)
    sin = np.ascontiguousarray(
        (np.repeat(fc[:S, :, 1], 2, axis=1) * sgn[None, :]).T
    ).astype(BF16)
    k_idx = np.arange(128)[:, None]
    r_idx = np.arange(128)[None, :]
    tri = np.where(r_idx >= k_idx, 0.0, -1e9).astype(np.float32)

    in_maps = []
    for c in range(NCORE):
        s, h2 = c // 2, c % 2
        in_maps.append(
            {
                "xT": np.ascontiguousarray(xT[:, s * S:(s + 1) * S]),
                "wq": wq[:, h2 * HLOC * D:(h2 + 1) * HLOC * D].astype(BF16),
                "wk": wk[:, h2 * GLOC * D:(h2 + 1) * GLOC * D].astype(BF16),
                "wv": wv[:, h2 * GLOC * D:(h2 + 1) * GLOC * D].astype(BF16),
                "wo": wo[h2 * HLOC * D:(h2 + 1) * HLOC * D, :].astype(BF16),
                "cosT": cos,
                "sinT": sin,
                "tri": tri,
            }
        )

    if _PROGRAM is None:
        _PROGRAM = _build_program()

    res = run_bass_kernel_spmd(
        _PROGRAM, in_maps, core_ids=list(range(NCORE)), trace=_TRACE
    )
    LAST_RESULT = res

    out = np.empty((B * S, DIM), np.float32)
    for s in range(B):
        outT = (
            res.results[2 * s]["outT"].astype(np.float32)
            + res.results[2 * s + 1]["outT"].astype(np.float32)
        )
        out[s * S:(s + 1) * S, :] = outT.T
    return out
